# revision 10
# baseline (speedup 1.0000x reference)
"""Locally-connected 2D layer on 8 Trainium2 NeuronCores.

Problem: x[128,3,64,64] f32, per-position weights W[60,60,32,75], bias b[60,60,32]
  out[b,o,y,x] = sum_k patches[b,y,x,k] * W[y,x,o,k] + b[y,x,o],  k=(c,dy,dx)

Strategy (spatial sharding over output rows, 8 rows/core, memory-regime):
  - 4 output rows are computed per matmul ("super-row"): the mod-8 ring holds
    8 input rows as planes on partitions p = plane*8 + slot (plane=c*5+dx in
    [0,15), slot in [0,8)) -> 120 partitions, +1 ones row at p=120 for the
    bias. lhsT = W[121, 128] packs (k,o) for 4 rows; each k's weights sit on
    its own 75 active plane-partitions (zero elsewhere). One full-array
    [121,128]x[121,128] matmul per x-position -> out[(k,o), b] in PSUM.
    128-col weights enable the compiler's fast-weight-load path.
  - Per-row dy rotation is folded into the host-side W layout (slot =
    (row+dy) mod 8), so the device reads partitions 0:121 always.
  - The x range is split in two SBUF tiles (xpA: x<32, xpB: x>=32) so refill
    DMAs and matmul reads of different halves never need write-write
    ordering. Slot refills (input rows 8..11, between the two super-rows)
    write 15 stride-8 partitions -> spread over ~15 DMA engines.
  - All DMAs on the two HWDGE queues: sync = patch fills + refills,
    scalar = W preloads + output stores. Everything chunked along x so the
    first matmul starts after ~0.5MB of input instead of 4MB.
  - PSUM chunks of 8 x-positions ([128, 1024] f32, 2 banks); drains
    alternate DVE / ACT, casting f32->bf16; one store per chunk.
"""

import numpy as np

B, C, H, WIDTH = 128, 3, 64, 64
KH = KW = 5
RY = RX = 60
O = 32
K = 75
NCORES = 8
RPC = 8             # output rows per core (8*8=64, last 4 dropped)
SR = 2              # super-rows per core (4 output rows each)
INR = RPC + KH - 1  # 12 input rows per core
PADH = NCORES * RPC + KH - 1  # 68
FXB = RX * B        # 7680 elements per patch plane
NPL = KH * C        # 15 planes per input row
NSL = 8             # ring slots
KP = NPL * NSL + 1  # 121 contraction partitions (120 planes + ones)
XA = 32             # x-positions in tile A
FA = XA * B         # 4096
FB = FXB - FA       # 3584
# PSUM chunks, grouped into two half-super-row output tiles (x 0-30, 30-60)
CHUNKS = ((0, 8), (8, 16), (16, 24), (24, 30),
          (30, 38), (38, 46), (46, 54), (54, 60))

_cache = {}


def _build():
    import concourse.bass as bass
    import concourse.bacc as bacc
    import concourse.tile as tile
    import concourse.mybir as mybir

    f32 = mybir.dt.float32
    bf16 = mybir.dt.bfloat16
    nc = bacc.Bacc("TRN2", target_bir_lowering=False, debug=False,
                   num_devices=NCORES)
    xpr_d = nc.dram_tensor("xpr", [INR, NPL, FXB], bf16, kind="ExternalInput")
    # wh[sr, p, x, k, o]
    wh_d = nc.dram_tensor("wh", [SR, KP, RX, 4, O], bf16, kind="ExternalInput")
    ones_d = nc.dram_tensor("ones", [1, FXB], bf16, kind="ExternalInput")
    # oc[sr, (k,o), x, b]
    oc_d = nc.dram_tensor("oc", [SR, 4 * O, RX, B], bf16, kind="ExternalOutput")

    with tile.TileContext(nc) as tc:
        with (
            tc.tile_pool(name="const", bufs=1) as cpool,
            tc.tile_pool(name="os", bufs=4) as opool,
            tc.tile_pool(name="ps", bufs=3, space=bass.MemorySpace.PSUM) as ppool,
        ):
            xpA = cpool.tile([KP, FA], bf16)
            xpB = cpool.tile([KP, FB], bf16)
            wts = [cpool.tile([KP, RX * 4 * O], bf16, name=f"wt{sr}")
                   for sr in range(SR)]

            nc.sync.dma_start(xpA[KP - 1:KP, :], ones_d[:, :FA])
            nc.sync.dma_start(xpB[KP - 1:KP, :], ones_d[:, FA:])
            # initial ring fill (input rows 0..7); src iterates
            # (plane, row, f) to match p = plane*NSL + row. First chunk
            # small so the first matmul starts early.
            for x0, x1 in ((0, 8), (8, 32)):
                nc.sync.dma_start(
                    xpA[:KP - 1, x0 * B:x1 * B],
                    xpr_d[0:NSL, :, x0 * B:x1 * B].transpose([1, 0, 2]))
            for x0, x1 in ((32, 46), (46, 60)):
                nc.sync.dma_start(
                    xpB[:KP - 1, (x0 - XA) * B:(x1 - XA) * B],
                    xpr_d[0:NSL, :, x0 * B:x1 * B].transpose([1, 0, 2]))

            # W preloads: wt0 chunked (it gates startup), wt1 on sync
            for x0, x1 in ((0, 8), (8, 32), (32, 60)):
                nc.scalar.dma_start(wts[0][:, x0 * 128:x1 * 128],
                                    wh_d[0, :, x0:x1])
            nc.sync.dma_start(wts[1][:, :], wh_d[1, :, :])

            for sr in range(SR):
                ots = [opool.tile([128, 30 * B], bf16, name="ot") for _ in range(2)]
                for ci, (x0, x1) in enumerate(CHUNKS):
                    nx = x1 - x0
                    pt = ppool.tile([128, 8 * B], f32, name="pt", tag="pt")
                    for xi in range(nx):
                        x = x0 + xi
                        src = (xpA[:, x * B:(x + 1) * B] if x < XA else
                               xpB[:, (x - XA) * B:(x - XA + 1) * B])
                        nc.tensor.matmul(
                            pt[:, xi * B:(xi + 1) * B],
                            wts[sr][:, x * 128:(x + 1) * 128],
                            src,
                            tile_position=(0, 0),
                        )
                    if sr == 0 and x1 == 38:
                        # all x<32 matmuls of sr0 done -> refill ring slots
                        # 0..3 (input rows 8..11) in the A half
                        for s in range(INR - NSL):
                            nc.sync.dma_start(xpA[s:KP - 1:NSL, :],
                                              xpr_d[NSL + s, :, :FA])
                    if sr == 0 and x1 == RX:
                        for s in range(INR - NSL):
                            nc.sync.dma_start(xpB[s:KP - 1:NSL, :],
                                              xpr_d[NSL + s, :, FA:])
                    ot = ots[ci // 4]
                    c0 = (x0 - (0 if ci < 4 else 30)) * B
                    if ci % 2 == 0:
                        nc.vector.tensor_copy(ot[:, c0:c0 + nx * B],
                                              pt[:, :nx * B])
                    else:
                        nc.scalar.copy(ot[:, c0:c0 + nx * B], pt[:, :nx * B])
                    if ci % 4 == 3:
                        h0 = 0 if ci < 4 else 30
                        nc.scalar.dma_start(
                            oc_d[sr, :, h0:h0 + 30, :].rearrange(
                                "p x b -> p (x b)"), ot[:])

    nc.compile()
    return nc


def _get_nc():
    if "nc" not in _cache:
        _cache["nc"] = _build()
    return _cache["nc"]


def _prep_inputs(x, W, b):
    import ml_dtypes
    bf = ml_dtypes.bfloat16

    x = np.asarray(x, np.float32)
    W = np.asarray(W, np.float32)
    b = np.asarray(b, np.float32)
    xh = np.zeros((PADH, C, WIDTH, B), np.float32)
    xh[:H] = x.transpose(2, 1, 3, 0)  # [row, c, w, batch]
    # patch planes: xpr_full[r, plane = c*KW+dx, x*B+b] = xh[r, c, x+dx, b]
    xpr_full = np.zeros((PADH, C, KW, RX, B), np.float32)
    for dx in range(KW):
        xpr_full[:, :, dx] = xh[:, :, dx:dx + RX]
    xpr_full = xpr_full.reshape(PADH, NPL, FXB).astype(bf)

    # W partition map: (c, dy, dx) -> p = (c*KW+dx)*NSL + (row+dy)%NSL
    cidx = np.arange(C)[:, None, None]
    dyidx = np.arange(KH)[None, :, None]
    dxidx = np.arange(KW)[None, None, :]
    in_maps = []
    for i in range(NCORES):
        whc = np.zeros((SR, KP, RX, 4, O), np.float32)
        for sr in range(SR):
            for k in range(4):
                r = sr * 4 + k
                y = RPC * i + r
                if y < RY:
                    # W[y]: [RX, O, 75] with kidx = c*25 + dy*5 + dx
                    wy = W[y].reshape(RX, O, C, KH, KW).transpose(2, 3, 4, 0, 1)
                    pidx = ((cidx * KW + dxidx) * NSL + (r + dyidx) % NSL)
                    whc[sr, pidx.reshape(-1), :, k, :] = wy.reshape(K, RX, O)
                    whc[sr, KP - 1, :, k, :] = b[y]
        in_maps.append({
            "xpr": np.ascontiguousarray(xpr_full[RPC * i:RPC * i + INR]),
            "wh": whc.astype(bf),
            "ones": np.ones((1, FXB), bf),
        })
    return in_maps


def kernel(x, W, b):
    from concourse.bass_utils import run_bass_kernel_spmd

    nc = _get_nc()
    in_maps = _prep_inputs(x, W, b)
    br = run_bass_kernel_spmd(nc, in_maps, list(range(NCORES)),
                              **_cache.get("run_kwargs", {}))
    _cache["last_run"] = br
    oc = np.stack([np.asarray(br.results[i]["oc"]).astype(np.float32)
                   for i in range(NCORES)])
    # oc: [core, sr, (k,o), x, b] -> out[b, o, y= core*8+sr*4+k, x]
    oc = oc.reshape(NCORES * SR, 4, O, RX, B)
    out = oc.transpose(4, 2, 0, 1, 3).reshape(B, O, NCORES * RPC, RX)
    return np.ascontiguousarray(out[:, :, :RY, :])
